# revision 50
# baseline (speedup 1.0000x reference)
"""Multi-head attention Trainium2 kernel (8 NeuronCores).

Sharding: core c owns batch b = c//2 and heads h0 = (c%2)*6 .. h0+6 (tensor
parallel over heads x data parallel over batch). Each core computes its 6
heads' attention and a partial output projection; the host sums the two
partial projections per batch element and adds the output bias.

v2 design (vs f32r baseline):
  - all SBUF operands bf16 (DMA halved, DVE 2x eligible; PE cost unchanged)
  - V tiles per head pair laid out [V_a(64) | ones(64) | V_b(64)] so the
    ctx matmul produces 64 ctx rows + 64 replicated softmax denominators;
    normalize = reciprocal_approx_fast + one tensor_tensor per head
  - software-pipelined emission: scores(g+1) precedes ctx(g) in the PE
    queue so the PE never blocks behind ScalarE's exp
  - every QKV/O projection group is a "filler" pumped into the attention
    loop to absorb PE slack; dedicated PSUM pools (4+2+2 banks)
"""
import sys

sys.path.insert(0, "/opt/trn_rl_repo")

from contextlib import ExitStack

import ml_dtypes
import numpy as np

import concourse.bacc as bacc
import concourse.bass as bass
import concourse.mybir as mybir
import concourse.tile as tile
from concourse.bass_utils import run_bass_kernel_spmd

f32 = mybir.dt.float32
bf16 = mybir.dt.bfloat16
AF = mybir.ActivationFunctionType
ALU = mybir.AluOpType
BF = ml_dtypes.bfloat16

B, S, D = 4, 2048, 768
H, E = 12, 64
HL = 6              # heads per core
F = HL * E          # 384: local concat-head feature dim
ND = D // 128       # 6 contraction chunks over D
NF = F // 128       # 3 chunks over F (head pairs)
NK = S // 128       # 16 key chunks
QB = 512            # q block (matmul moving free dim)
NQB = S // QB       # 4
PW = 256            # per-pair V tile width [ones64 | V_a | ones64 | V_b]
VW = NF * PW        # 768
NCORES = 8

_NC = None


def _build(debug=False):
    nc = bacc.Bacc()
    xt_d = nc.declare_dram_parameter("xt", [D, S], bf16, isOutput=False)
    wq_d = nc.declare_dram_parameter("wq", [D, F], bf16, isOutput=False)
    wk_d = nc.declare_dram_parameter("wk", [D, F], bf16, isOutput=False)
    wv_d = nc.declare_dram_parameter("wv", [D, F], bf16, isOutput=False)
    wo_d = nc.declare_dram_parameter("wo", [F, D], bf16, isOutput=False)
    bq_d = nc.declare_dram_parameter("bq", [F, 1], f32, isOutput=False)
    bk_d = nc.declare_dram_parameter("bk", [F, 1], f32, isOutput=False)
    bv_d = nc.declare_dram_parameter("bv", [1, F], f32, isOutput=False)
    y_d = nc.declare_dram_parameter("y", [S, D], bf16, isOutput=True)

    with tile.TileContext(nc) as tc, ExitStack() as ctx:
        xpool = ctx.enter_context(tc.tile_pool(name="xpool", bufs=ND))
        qpool = ctx.enter_context(tc.tile_pool(name="qpool", bufs=NF))
        kpool = ctx.enter_context(tc.tile_pool(name="kpool", bufs=NF))
        cxpool = ctx.enter_context(tc.tile_pool(name="cxpool", bufs=NF))
        vpool = ctx.enter_context(tc.tile_pool(name="vpool", bufs=NK))
        wpool = ctx.enter_context(tc.tile_pool(name="wpool", bufs=3 * ND))
        wopool = ctx.enter_context(tc.tile_pool(name="wopool", bufs=NF))
        epool = ctx.enter_context(tc.tile_pool(name="epool", bufs=6))
        rpool = ctx.enter_context(tc.tile_pool(name="rpool", bufs=2))
        opool = ctx.enter_context(tc.tile_pool(name="opool", bufs=2))
        cpool = ctx.enter_context(tc.tile_pool(name="cpool", bufs=3))
        pssc = ctx.enter_context(tc.tile_pool(name="pssc", bufs=2, space="PSUM"))
        psctx = ctx.enter_context(tc.tile_pool(name="psctx", bufs=2, space="PSUM"))
        pproj = ctx.enter_context(tc.tile_pool(name="pproj", bufs=2, space="PSUM"))

        # --- bias tiles ---
        bq_sb = cpool.tile([128, NF], f32, name="bq_sb", tag="bq")
        nc.sync.dma_start(out=bq_sb, in_=bq_d.rearrange("(m p) o -> p m o", p=128))
        bk_sb = cpool.tile([128, NF], f32, name="bk_sb", tag="bk")
        nc.sync.dma_start(out=bk_sb, in_=bk_d.rearrange("(m p) o -> p m o", p=128))
        # bv broadcast across partitions via 0-stride DRAM read
        bv_bc = cpool.tile([128, F], f32, name="bv_bc", tag="bv")
        bv_src = bv_d[0:1, :]
        bv_ap = bass.AP(tensor=bv_src.tensor, offset=bv_src.offset,
                        ap=[[0, 128]] + list(bv_src.ap)[1:])
        nc.scalar.dma_start(out=bv_bc, in_=bv_ap)

        # --- input/weight DMAs: per-chunk 2D descriptors, issued in the
        # order compute consumes them (xt/wk pairs first, wo last)
        engs = (nc.sync, nc.scalar, nc.gpsimd)
        xt_t = [xpool.tile([128, S], bf16, tag="x", name=f"xt{kd}")
                for kd in range(ND)]
        wk_t = [wpool.tile([128, F], bf16, tag="w", name=f"wk{kd}")
                for kd in range(ND)]
        wq_t = [wpool.tile([128, F], bf16, tag="w", name=f"wq{kd}")
                for kd in range(ND)]
        wv_t = [wpool.tile([128, F], bf16, tag="w", name=f"wv{kd}")
                for kd in range(ND)]
        wo_t = [wopool.tile([128, D], bf16, tag="wo", name=f"wo{kf}")
                for kf in range(NF)]
        # xt/wk/wq split over sync+scalar queues (needed first);
        # wv/wo go on gpsimd BEHIND the v-ones memsets, deferring their
        # ring bandwidth until the early tensors have landed
        for kd in range(ND):
            eng = engs[kd % 2]
            eng.dma_start(out=xt_t[kd], in_=xt_d[kd * 128:(kd + 1) * 128, :])
            eng.dma_start(out=wk_t[kd], in_=wk_d[kd * 128:(kd + 1) * 128, :])
        for kd in range(ND):
            engs[kd % 2].dma_start(out=wq_t[kd],
                                   in_=wq_d[kd * 128:(kd + 1) * 128, :])

        # --- static tiles: V (with ones blocks), Q^T, K^T (pairs stacked) ---
        v_t = []
        for mk in range(NK):
            t = vpool.tile([128, VW], bf16, tag="v", name=f"v{mk}")
            t4 = t[:].rearrange("p (pr two c) -> p pr two c", two=2, c=128)
            nc.gpsimd.memset(t4[:, :, :, 0:E], 1.0)
            v_t.append(t)
        for kd in range(ND):
            nc.gpsimd.dma_start(out=wv_t[kd],
                                in_=wv_d[kd * 128:(kd + 1) * 128, :])
        for kf in range(NF):
            nc.gpsimd.dma_start(out=wo_t[kf],
                                in_=wo_d[kf * 128:(kf + 1) * 128, :])
        # qt: 2 tiles per pair, complementary head halves zeroed so scores can
        # run a full K=128 contraction (kt holds both heads' dims stacked)
        qt_t = [qpool.tile([128, S], bf16, tag="q", name=f"qt{h}")
                for h in range(2 * NF)]
        kt_t = [kpool.tile([128, S], bf16, tag="k", name=f"kt{m}")
                for m in range(NF)]
        ctxt_t = [cxpool.tile([128, S], bf16, tag="cx", name=f"ctxt{m}")
                  for m in range(NF)]
        for m in range(NF):
            nc.vector.memset(qt_t[2 * m][E:128, :], 0.0)
            nc.vector.memset(qt_t[2 * m + 1][0:E, :], 0.0)

        # --- projection group emitters (each: 6 matmuls + DVE drain) ---
        def emit_k_group(m, nq):
            ps = pproj.tile([128, QB], f32, tag="pj", name=f"pk{m}_{nq}")
            for kd in range(ND):
                nc.tensor.matmul(
                    ps[:, :],
                    lhsT=wk_t[kd][:, m * 128:(m + 1) * 128],
                    rhs=xt_t[kd][:, nq * QB:(nq + 1) * QB],
                    start=(kd == 0), stop=(kd == ND - 1),
                )
            nc.vector.tensor_scalar_add(
                kt_t[m][:, nq * QB:(nq + 1) * QB], ps[:, :], bk_sb[:, m:m + 1])

        def emit_q_group(m, nq):
            sl = slice(nq * QB, (nq + 1) * QB)
            ps = pproj.tile([128, QB], f32, tag="pj", name=f"pq{m}_{nq}")
            for kd in range(ND):
                nc.tensor.matmul(
                    ps[:, :],
                    lhsT=wq_t[kd][:, m * 128:(m + 1) * 128],
                    rhs=xt_t[kd][:, nq * QB:(nq + 1) * QB],
                    start=(kd == 0), stop=(kd == ND - 1),
                )
            nc.vector.tensor_scalar_add(
                qt_t[2 * m][0:E, sl], ps[0:E, :], bq_sb[0:E, m:m + 1])
            nc.vector.tensor_scalar_add(
                qt_t[2 * m + 1][E:128, sl], ps[E:128, :], bq_sb[E:128, m:m + 1])

        def emit_v_group(mk):
            ps = pproj.tile([128, F], f32, tag="pj", name=f"pv{mk}",
                            padded_shape=[128, QB])
            for kd in range(ND):
                nc.tensor.matmul(
                    ps[:, :],
                    lhsT=xt_t[kd][:, mk * 128:(mk + 1) * 128],
                    rhs=wv_t[kd][:, :],
                    start=(kd == 0), stop=(kd == ND - 1),
                )
            t4 = v_t[mk][:].rearrange("p (pr two c) -> p pr two c", two=2, c=128)
            ps3 = ps.rearrange("p (pr hh e) -> p pr hh e", hh=2, e=E)
            bv3 = bv_bc.rearrange("p (pr hh e) -> p pr hh e", hh=2, e=E)
            nc.vector.tensor_tensor(
                t4[:, :, 0, E:128], ps3[:, :, 0, :], bv3[:, :, 0, :], op=ALU.add)
            nc.vector.tensor_tensor(
                t4[:, :, 1, E:128], ps3[:, :, 1, :], bv3[:, :, 1, :], op=ALU.add)

        def emit_o_unit(mq):
            osb = opool.tile([128, D], bf16, tag="o", name=f"ot{mq}")
            for piece, (c0, c1) in enumerate(((0, 512), (512, D))):
                ps = pproj.tile([128, c1 - c0], f32, tag="pj",
                                name=f"po{piece}_{mq}", padded_shape=[128, QB])
                for kf in range(NF):
                    nc.tensor.matmul(
                        ps[:, :], lhsT=ctxt_t[kf][:, mq * 128:(mq + 1) * 128],
                        rhs=wo_t[kf][:, c0:c1],
                        start=(kf == 0), stop=(kf == NF - 1))
                nc.vector.tensor_copy(osb[:, c0:c1], ps[:, :])
            eng = (nc.sync, nc.scalar, nc.gpsimd)[mq % 3]
            eng.dma_start(out=y_d[mq * 128:(mq + 1) * 128, :], in_=osb[:])

        fillers = []
        end_fillers = []  # reserved: one popped at each unit's tail stall

        def pump(n=1):
            for _ in range(n):
                if fillers:
                    fillers.pop(0)()

        def pump_end():
            if end_fillers:
                end_fillers.pop(0)()
            else:
                pump(1)

        # --- upfront: only what unit (0,0)'s first scores need ---
        emit_k_group(0, 0)
        emit_q_group(0, 0)

        # --- attention unit: head pair hp, q block nq ---
        # slots: g -> number of fillers to pump after that g's scores
        # (8 = the pump point just before ctx(7)); spreading pumps across
        # the unit keeps PE fed at the exp(7)->ctx(7) tail stall
        def unit(hp, nq, slots=None):
            if slots is None:
                slots = {1: 1, 3: 1, 5: 1, 7: 1}
            qsl = slice(nq * QB, (nq + 1) * QB)
            cps = [psctx.tile([128, QB], f32, tag="c", name=f"c{hp}_{nq}_{h}")
                   for h in range(2)]
            sps = [None, None]
            eb = [None, None]
            prev = [None, None]  # previous g's (esb, sps-group) per head

            def emit_scores(g):
                for hh in range(2):
                    sps[hh] = pssc.tile([128, 2 * QB], f32, tag="s",
                                        name=f"s{hp}_{nq}_{g}_{hh}")
                for j in range(2):
                    mk = 2 * g + j
                    for hh in range(2):
                        nc.tensor.matmul(
                            sps[hh][:, j * QB:(j + 1) * QB],
                            lhsT=kt_t[hp][:, mk * 128:(mk + 1) * 128],
                            rhs=qt_t[2 * hp + hh][:, qsl],
                            start=True, stop=True,
                        )

            def emit_exp(g):
                for hh in range(2):
                    e = epool.tile([128, 2 * QB], bf16, tag="e",
                                   name=f"e{hp}_{nq}_{g}_{hh}")
                    if g == 7:
                        # split halves so ctx(7) j=0 can start sooner
                        nc.scalar.activation(e[:, 0:QB], sps[hh][:, 0:QB],
                                             AF.Exp, scale=0.125)
                        nc.scalar.activation(e[:, QB:2 * QB], sps[hh][:, QB:2 * QB],
                                             AF.Exp, scale=0.125)
                    else:
                        nc.scalar.activation(e[:], sps[hh][:, :], AF.Exp,
                                             scale=0.125)
                    eb[hh] = e

            def emit_ctx(g, ebs):
                for hh in range(2):
                    base = hp * PW + hh * 128  # [ones64 | V_h]
                    for j in range(2):
                        mk = 2 * g + j
                        nc.tensor.matmul(
                            cps[hh][:, :],
                            lhsT=v_t[mk][:, base:base + 128],
                            rhs=ebs[hh][:, j * QB:(j + 1) * QB],
                            start=(g == 0 and j == 0),
                            stop=(g == 7 and j == 1),
                        )

            for g in range(8):
                emit_scores(g)
                pump(slots.get(g, 0))
                if g > 0:
                    emit_ctx(g - 1, prev)
                emit_exp(g)
                prev = list(eb)
            pump(slots.get(8, 0))
            pump_end()
            # g=7 tail: per head finish ctx, then immediately normalize so
            # the DVE recip+mult chain overlaps the other head's ctx matmuls
            # (both heads produce [den 0:64 | ctx 64:128])
            for hh in range(2):
                base = hp * PW + hh * 128
                for j in range(2):
                    nc.tensor.matmul(
                        cps[hh][:, :],
                        lhsT=v_t[14 + j][:, base:base + 128],
                        rhs=prev[hh][:, j * QB:(j + 1) * QB],
                        start=False, stop=(j == 1),
                    )
                r = rpool.tile([E, QB], f32, tag="r", name=f"r{hp}_{nq}_{hh}")
                nc.vector.reciprocal_approx_fast(out=r[:], in_=cps[hh][0:E, :])
                nc.vector.tensor_tensor(
                    ctxt_t[hp][hh * E:(hh + 1) * E, qsl], cps[hh][E:128, :],
                    r[:], op=ALU.mult)

        # --- schedule: (regular fillers, end filler) per unit ---
        K, Q, V, O = emit_k_group, emit_q_group, emit_v_group, emit_o_unit
        plan = {
            (0, 0): ([lambda: K(0, 1), lambda: V(0), lambda: V(1),
                      lambda: K(0, 2), lambda: K(0, 3), lambda: Q(0, 1)]
                     + [lambda m=m: V(m) for m in range(2, 15)],
                     [lambda: V(15)]),
            (0, 1): ([lambda: Q(0, 2)], [lambda: K(1, 0)]),
            (0, 2): ([lambda: Q(0, 3)], [lambda: K(1, 1)]),
            (0, 3): ([lambda: K(1, 2), lambda: Q(1, 0)], [lambda: K(1, 3)]),
            (1, 0): ([lambda: Q(1, 1)], [lambda: K(2, 0)]),
            (1, 1): ([lambda: Q(1, 2)], [lambda: K(2, 1)]),
            (1, 2): ([lambda: Q(1, 3)], [lambda: K(2, 2)]),
            (1, 3): ([lambda: Q(2, 0)], [lambda: K(2, 3)]),
            (2, 0): ([lambda: Q(2, 1)], [lambda: Q(2, 2)]),
            (2, 1): ([lambda m=m: O(m) for m in range(0, 3)], [lambda: O(3)]),
            (2, 2): ([lambda: Q(2, 3)] + [lambda m=m: O(m) for m in range(4, 7)],
                     [lambda: O(7)]),
            (2, 3): ([lambda m=m: O(m) for m in range(8, 11)], [lambda: O(11)]),
        }
        full = {0: 3, 1: 3, 2: 2, 3: 2, 4: 2, 5: 2, 6: 2, 7: 2, 8: 2}
        for hp in range(NF):
            for nq in range(NQB):
                reg, end = plan[(hp, nq)]
                fillers.extend(reg)
                end_fillers.extend(end)
                unit(hp, nq, slots=full if (hp, nq) == (0, 0) else None)
        # drain remaining fillers, then tail output projections
        while fillers or end_fillers:
            pump(1)
            pump_end()
        for mq in range(12, 16):
            emit_o_unit(mq)
        if debug:
            qt_dbg = nc.declare_dram_parameter("qt_dbg", [2 * NF * 128, S], bf16,
                                               isOutput=True)
            kt_dbg = nc.declare_dram_parameter("kt_dbg", [NF * 128, S], bf16,
                                               isOutput=True)
            v_dbg = nc.declare_dram_parameter("v_dbg", [NK * 128, VW], bf16,
                                              isOutput=True)
            cx_dbg = nc.declare_dram_parameter("cx_dbg", [NF * 128, S], bf16,
                                               isOutput=True)
            for h in range(2 * NF):
                nc.sync.dma_start(out=qt_dbg[h * 128:(h + 1) * 128, :], in_=qt_t[h][:])
            for m in range(NF):
                nc.sync.dma_start(out=kt_dbg[m * 128:(m + 1) * 128, :], in_=kt_t[m][:])
                nc.sync.dma_start(out=cx_dbg[m * 128:(m + 1) * 128, :], in_=ctxt_t[m][:])
            for mk in range(NK):
                nc.sync.dma_start(out=v_dbg[mk * 128:(mk + 1) * 128, :], in_=v_t[mk][:])
    nc.compile()
    return nc


def _get_nc():
    global _NC
    if _NC is None:
        _NC = _build()
    return _NC


def kernel(x, Wq, bq, Wk, bk, Wv, bv, Wo, bo, _trace=False):
    x = np.asarray(x, dtype=np.float32)
    Wq = np.asarray(Wq, dtype=np.float32)
    bq = np.asarray(bq, dtype=np.float32)
    Wk = np.asarray(Wk, dtype=np.float32)
    bk = np.asarray(bk, dtype=np.float32)
    Wv = np.asarray(Wv, dtype=np.float32)
    bv = np.asarray(bv, dtype=np.float32)
    Wo = np.asarray(Wo, dtype=np.float32)
    bo = np.asarray(bo, dtype=np.float32)

    nc = _get_nc()
    in_maps = []
    for c in range(NCORES):
        b = c // 2
        h0 = (c % 2) * HL
        in_maps.append({
            "xt": np.ascontiguousarray(x[b].T).astype(BF),
            "wq": np.ascontiguousarray(
                Wq[h0:h0 + HL].transpose(1, 0, 2).reshape(D, F)).astype(BF),
            "wk": np.ascontiguousarray(
                Wk[h0:h0 + HL].transpose(1, 0, 2).reshape(D, F)).astype(BF),
            "wv": np.ascontiguousarray(
                Wv[h0:h0 + HL].transpose(1, 0, 2).reshape(D, F)).astype(BF),
            "wo": np.ascontiguousarray(Wo[h0 * E:(h0 + HL) * E]).astype(BF),
            "bq": np.ascontiguousarray(bq[h0:h0 + HL].reshape(F, 1)),
            "bk": np.ascontiguousarray(bk[h0:h0 + HL].reshape(F, 1)),
            "bv": np.ascontiguousarray(bv[h0:h0 + HL].reshape(1, F)),
        })
    res = run_bass_kernel_spmd(nc, in_maps, list(range(NCORES)), trace=_trace)
    out = np.empty((B, S, D), np.float32)
    for b in range(B):
        out[b] = (res.results[2 * b]["y"].astype(np.float32)
                  + res.results[2 * b + 1]["y"].astype(np.float32)
                  + bo[None, :])
    if _trace:
        kernel.last_exec_time_ns = res.exec_time_ns
        kernel.last_results = res
    return out


# revision 53
# speedup vs baseline: 1.2014x; 1.2014x over previous
"""Multi-head attention Trainium2 kernel (8 NeuronCores).

Sharding: core c owns batch b = c//2 and heads h0 = (c%2)*6 .. h0+6 (tensor
parallel over heads x data parallel over batch). Each core computes its 6
heads' attention and a partial output projection; the host sums the two
partial projections per batch element and adds the output bias.

v2 design (vs f32r baseline):
  - all SBUF operands bf16 (DMA halved, DVE 2x eligible; PE cost unchanged)
  - V tiles per head pair laid out [V_a(64) | ones(64) | V_b(64)] so the
    ctx matmul produces 64 ctx rows + 64 replicated softmax denominators;
    normalize = reciprocal_approx_fast + one tensor_tensor per head
  - software-pipelined emission: scores(g+1) precedes ctx(g) in the PE
    queue so the PE never blocks behind ScalarE's exp
  - every QKV/O projection group is a "filler" pumped into the attention
    loop to absorb PE slack; dedicated PSUM pools (4+2+2 banks)
"""
import sys

sys.path.insert(0, "/opt/trn_rl_repo")

from contextlib import ExitStack

import ml_dtypes
import numpy as np

import concourse.bacc as bacc
import concourse.bass as bass
import concourse.mybir as mybir
import concourse.tile as tile
from concourse.bass_utils import run_bass_kernel_spmd

f32 = mybir.dt.float32
bf16 = mybir.dt.bfloat16
AF = mybir.ActivationFunctionType
ALU = mybir.AluOpType
BF = ml_dtypes.bfloat16

B, S, D = 4, 2048, 768
H, E = 12, 64
HL = 6              # heads per core
F = HL * E          # 384: local concat-head feature dim
ND = D // 128       # 6 contraction chunks over D
NF = F // 128       # 3 chunks over F (head pairs)
NK = S // 128       # 16 key chunks
QB = 512            # q block (matmul moving free dim)
NQB = S // QB       # 4
PW = 256            # per-pair V tile width [ones64 | V_a | ones64 | V_b]
VW = NF * PW        # 768
NCORES = 8

_NC = None


def _build(debug=False):
    nc = bacc.Bacc()
    xt_d = nc.declare_dram_parameter("xt", [D, S], bf16, isOutput=False)
    wq_d = nc.declare_dram_parameter("wq", [D, F], bf16, isOutput=False)
    wk_d = nc.declare_dram_parameter("wk", [D, F], bf16, isOutput=False)
    wv_d = nc.declare_dram_parameter("wv", [D, F], bf16, isOutput=False)
    wo_d = nc.declare_dram_parameter("wo", [F, D], bf16, isOutput=False)
    bq_d = nc.declare_dram_parameter("bq", [F, 1], f32, isOutput=False)
    bk_d = nc.declare_dram_parameter("bk", [F, 1], f32, isOutput=False)
    bv_d = nc.declare_dram_parameter("bv", [1, F], f32, isOutput=False)
    y_d = nc.declare_dram_parameter("y", [S, D], bf16, isOutput=True)

    with tile.TileContext(nc) as tc, ExitStack() as ctx:
        xpool = ctx.enter_context(tc.tile_pool(name="xpool", bufs=ND))
        qpool = ctx.enter_context(tc.tile_pool(name="qpool", bufs=NF))
        kpool = ctx.enter_context(tc.tile_pool(name="kpool", bufs=NF))
        cxpool = ctx.enter_context(tc.tile_pool(name="cxpool", bufs=NF))
        vpool = ctx.enter_context(tc.tile_pool(name="vpool", bufs=NK))
        wpool = ctx.enter_context(tc.tile_pool(name="wpool", bufs=3 * ND))
        wopool = ctx.enter_context(tc.tile_pool(name="wopool", bufs=NF))
        epool = ctx.enter_context(tc.tile_pool(name="epool", bufs=6))
        rpool = ctx.enter_context(tc.tile_pool(name="rpool", bufs=2))
        opool = ctx.enter_context(tc.tile_pool(name="opool", bufs=2))
        cpool = ctx.enter_context(tc.tile_pool(name="cpool", bufs=3))
        pssc = ctx.enter_context(tc.tile_pool(name="pssc", bufs=2, space="PSUM"))
        psctx = ctx.enter_context(tc.tile_pool(name="psctx", bufs=2, space="PSUM"))
        pproj = ctx.enter_context(tc.tile_pool(name="pproj", bufs=2, space="PSUM"))

        # --- bias tiles ---
        bq_sb = cpool.tile([128, NF], f32, name="bq_sb", tag="bq")
        nc.sync.dma_start(out=bq_sb, in_=bq_d.rearrange("(m p) o -> p m o", p=128))
        bk_sb = cpool.tile([128, NF], f32, name="bk_sb", tag="bk")
        nc.sync.dma_start(out=bk_sb, in_=bk_d.rearrange("(m p) o -> p m o", p=128))
        # bv broadcast across partitions via 0-stride DRAM read
        bv_bc = cpool.tile([128, F], f32, name="bv_bc", tag="bv")
        bv_src = bv_d[0:1, :]
        bv_ap = bass.AP(tensor=bv_src.tensor, offset=bv_src.offset,
                        ap=[[0, 128]] + list(bv_src.ap)[1:])
        nc.scalar.dma_start(out=bv_bc, in_=bv_ap)

        # --- input/weight DMAs: per-chunk 2D descriptors, issued in the
        # order compute consumes them (xt/wk pairs first, wo last)
        engs = (nc.sync, nc.scalar, nc.gpsimd)
        xt_t = [xpool.tile([128, S], bf16, tag="x", name=f"xt{kd}")
                for kd in range(ND)]
        wk_t = [wpool.tile([128, F], bf16, tag="w", name=f"wk{kd}")
                for kd in range(ND)]
        wq_t = [wpool.tile([128, F], bf16, tag="w", name=f"wq{kd}")
                for kd in range(ND)]
        wv_t = [wpool.tile([128, F], bf16, tag="w", name=f"wv{kd}")
                for kd in range(ND)]
        wo_t = [wopool.tile([128, D], bf16, tag="wo", name=f"wo{kf}")
                for kf in range(NF)]
        # xt/wk/wq split over sync+scalar queues (needed first);
        # wv/wo go on gpsimd BEHIND the v-ones memsets, deferring their
        # ring bandwidth until the early tensors have landed
        for kd in range(ND):
            eng = engs[kd % 2]
            eng.dma_start(out=xt_t[kd], in_=xt_d[kd * 128:(kd + 1) * 128, :])
            eng.dma_start(out=wk_t[kd], in_=wk_d[kd * 128:(kd + 1) * 128, :])
        for kd in range(ND):
            engs[kd % 2].dma_start(out=wq_t[kd],
                                   in_=wq_d[kd * 128:(kd + 1) * 128, :])

        # --- static tiles: V (with ones blocks), Q^T, K^T (pairs stacked) ---
        v_t = []
        for mk in range(NK):
            t = vpool.tile([128, VW], bf16, tag="v", name=f"v{mk}")
            t4 = t[:].rearrange("p (pr two c) -> p pr two c", two=2, c=128)
            nc.gpsimd.memset(t4[:, :, :, 0:E], 1.0)
            v_t.append(t)
        for kd in range(ND):
            nc.gpsimd.dma_start(out=wv_t[kd],
                                in_=wv_d[kd * 128:(kd + 1) * 128, :])
        for kf in range(NF):
            nc.gpsimd.dma_start(out=wo_t[kf],
                                in_=wo_d[kf * 128:(kf + 1) * 128, :])
        # qt: 2 tiles per pair, complementary head halves zeroed so scores can
        # run a full K=128 contraction (kt holds both heads' dims stacked)
        qt_t = [qpool.tile([128, S], bf16, tag="q", name=f"qt{h}")
                for h in range(2 * NF)]
        kt_t = [kpool.tile([128, S], bf16, tag="k", name=f"kt{m}")
                for m in range(NF)]
        ctxt_t = [cxpool.tile([128, S], bf16, tag="cx", name=f"ctxt{m}")
                  for m in range(NF)]
        for m in range(NF):
            nc.vector.memset(qt_t[2 * m][E:128, :], 0.0)
            nc.vector.memset(qt_t[2 * m + 1][0:E, :], 0.0)

        # --- projection group emitters (each: 6 matmuls + DVE drain) ---
        def emit_k_group(m, nq):
            ps = pproj.tile([128, QB], f32, tag="pj", name=f"pk{m}_{nq}")
            for kd in range(ND):
                nc.tensor.matmul(
                    ps[:, :],
                    lhsT=wk_t[kd][:, m * 128:(m + 1) * 128],
                    rhs=xt_t[kd][:, nq * QB:(nq + 1) * QB],
                    start=(kd == 0), stop=(kd == ND - 1),
                )
            nc.vector.tensor_scalar_add(
                kt_t[m][:, nq * QB:(nq + 1) * QB], ps[:, :], bk_sb[:, m:m + 1])

        def emit_q_group(m, nq):
            sl = slice(nq * QB, (nq + 1) * QB)
            ps = pproj.tile([128, QB], f32, tag="pj", name=f"pq{m}_{nq}")
            for kd in range(ND):
                nc.tensor.matmul(
                    ps[:, :],
                    lhsT=wq_t[kd][:, m * 128:(m + 1) * 128],
                    rhs=xt_t[kd][:, nq * QB:(nq + 1) * QB],
                    start=(kd == 0), stop=(kd == ND - 1),
                )
            nc.vector.tensor_scalar_add(
                qt_t[2 * m][0:E, sl], ps[0:E, :], bq_sb[0:E, m:m + 1])
            nc.vector.tensor_scalar_add(
                qt_t[2 * m + 1][E:128, sl], ps[E:128, :], bq_sb[E:128, m:m + 1])

        def emit_v_group(mk):
            ps = pproj.tile([128, F], f32, tag="pj", name=f"pv{mk}",
                            padded_shape=[128, QB])
            for kd in range(ND):
                nc.tensor.matmul(
                    ps[:, :],
                    lhsT=xt_t[kd][:, mk * 128:(mk + 1) * 128],
                    rhs=wv_t[kd][:, :],
                    start=(kd == 0), stop=(kd == ND - 1),
                )
            t4 = v_t[mk][:].rearrange("p (pr two c) -> p pr two c", two=2, c=128)
            ps3 = ps.rearrange("p (pr hh e) -> p pr hh e", hh=2, e=E)
            bv3 = bv_bc.rearrange("p (pr hh e) -> p pr hh e", hh=2, e=E)
            nc.vector.tensor_tensor(
                t4[:, :, 0, E:128], ps3[:, :, 0, :], bv3[:, :, 0, :], op=ALU.add)
            nc.vector.tensor_tensor(
                t4[:, :, 1, E:128], ps3[:, :, 1, :], bv3[:, :, 1, :], op=ALU.add)

        def emit_o_unit(mq):
            osb = opool.tile([128, D], bf16, tag="o", name=f"ot{mq}")
            for piece, (c0, c1) in enumerate(((0, 512), (512, D))):
                ps = pproj.tile([128, c1 - c0], f32, tag="pj",
                                name=f"po{piece}_{mq}", padded_shape=[128, QB])
                for kf in range(NF):
                    nc.tensor.matmul(
                        ps[:, :], lhsT=ctxt_t[kf][:, mq * 128:(mq + 1) * 128],
                        rhs=wo_t[kf][:, c0:c1],
                        start=(kf == 0), stop=(kf == NF - 1))
                nc.vector.tensor_copy(osb[:, c0:c1], ps[:, :])
            eng = (nc.sync, nc.scalar, nc.gpsimd)[mq % 3]
            eng.dma_start(out=y_d[mq * 128:(mq + 1) * 128, :], in_=osb[:])

        fillers = []
        end_fillers = []  # reserved: one popped at each unit's tail stall

        def pump(n=1):
            for _ in range(n):
                if fillers:
                    fillers.pop(0)()

        def pump_end():
            if end_fillers:
                end_fillers.pop(0)()
            else:
                pump(1)

        # --- upfront: only what unit (0,0)'s first scores need ---
        emit_k_group(0, 0)
        emit_q_group(0, 0)

        # --- attention unit: head pair hp, q block nq ---
        # slots: g -> number of fillers to pump after that g's scores
        # (8 = the pump point just before ctx(7)); spreading pumps across
        # the unit keeps PE fed at the exp(7)->ctx(7) tail stall
        def unit(hp, nq, slots=None):
            if slots is None:
                slots = {1: 1, 3: 1, 5: 1, 7: 1}
            qsl = slice(nq * QB, (nq + 1) * QB)
            cps = [psctx.tile([128, QB], f32, tag="c", name=f"c{hp}_{nq}_{h}")
                   for h in range(2)]
            sps = [None, None]
            eb = [None, None]
            prev = [None, None]  # previous g's (esb, sps-group) per head

            def emit_scores(g):
                for hh in range(2):
                    sps[hh] = pssc.tile([128, 2 * QB], f32, tag="s",
                                        name=f"s{hp}_{nq}_{g}_{hh}")
                for j in range(2):
                    mk = 2 * g + j
                    for hh in range(2):
                        nc.tensor.matmul(
                            sps[hh][:, j * QB:(j + 1) * QB],
                            lhsT=kt_t[hp][:, mk * 128:(mk + 1) * 128],
                            rhs=qt_t[2 * hp + hh][:, qsl],
                            start=True, stop=True,
                        )

            def emit_exp(g):
                for hh in range(2):
                    e = epool.tile([128, 2 * QB], bf16, tag="e",
                                   name=f"e{hp}_{nq}_{g}_{hh}")
                    if g == 7:
                        # split halves so ctx(7) j=0 can start sooner
                        nc.scalar.activation(e[:, 0:QB], sps[hh][:, 0:QB],
                                             AF.Exp, scale=0.125)
                        nc.scalar.activation(e[:, QB:2 * QB], sps[hh][:, QB:2 * QB],
                                             AF.Exp, scale=0.125)
                    else:
                        nc.scalar.activation(e[:], sps[hh][:, :], AF.Exp,
                                             scale=0.125)
                    eb[hh] = e

            def emit_ctx(g, ebs):
                for hh in range(2):
                    base = hp * PW + hh * 128  # [ones64 | V_h]
                    for j in range(2):
                        mk = 2 * g + j
                        nc.tensor.matmul(
                            cps[hh][:, :],
                            lhsT=v_t[mk][:, base:base + 128],
                            rhs=ebs[hh][:, j * QB:(j + 1) * QB],
                            start=(g == 0 and j == 0),
                            stop=(g == 7 and j == 1),
                        )

            for g in range(8):
                emit_scores(g)
                pump(slots.get(g, 0))
                if g > 0:
                    emit_ctx(g - 1, prev)
                emit_exp(g)
                prev = list(eb)
            pump(slots.get(8, 0))
            pump_end()
            # g=7 tail: per head finish ctx, then immediately normalize so
            # the DVE recip+mult chain overlaps the other head's ctx matmuls
            # (both heads produce [den 0:64 | ctx 64:128])
            for hh in range(2):
                base = hp * PW + hh * 128
                for j in range(2):
                    nc.tensor.matmul(
                        cps[hh][:, :],
                        lhsT=v_t[14 + j][:, base:base + 128],
                        rhs=prev[hh][:, j * QB:(j + 1) * QB],
                        start=False, stop=(j == 1),
                    )
                r = rpool.tile([E, QB], f32, tag="r", name=f"r{hp}_{nq}_{hh}")
                nc.vector.reciprocal_approx_fast(out=r[:], in_=cps[hh][0:E, :])
                nc.vector.tensor_tensor(
                    ctxt_t[hp][hh * E:(hh + 1) * E, qsl], cps[hh][E:128, :],
                    r[:], op=ALU.mult)

        # --- schedule: (regular fillers, end filler) per unit ---
        K, Q, V, O = emit_k_group, emit_q_group, emit_v_group, emit_o_unit
        plan = {
            (0, 0): ([lambda: K(0, 1), lambda: V(0), lambda: V(1),
                      lambda: K(0, 2), lambda: K(0, 3), lambda: Q(0, 1)]
                     + [lambda m=m: V(m) for m in range(2, 15)],
                     [lambda: V(15)]),
            (0, 1): ([lambda: Q(0, 2)], [lambda: K(1, 0)]),
            (0, 2): ([lambda: Q(0, 3)], [lambda: K(1, 1)]),
            (0, 3): ([lambda: K(1, 2), lambda: Q(1, 0)], [lambda: K(1, 3)]),
            (1, 0): ([lambda: Q(1, 1)], [lambda: K(2, 0)]),
            (1, 1): ([lambda: Q(1, 2)], [lambda: K(2, 1)]),
            (1, 2): ([lambda: Q(1, 3)], [lambda: K(2, 2)]),
            (1, 3): ([lambda: Q(2, 0)], [lambda: K(2, 3)]),
            (2, 0): ([lambda: Q(2, 1)], [lambda: Q(2, 2)]),
            (2, 1): ([lambda m=m: O(m) for m in range(0, 3)], [lambda: O(3)]),
            (2, 2): ([lambda: Q(2, 3)] + [lambda m=m: O(m) for m in range(4, 7)],
                     [lambda: O(7)]),
            (2, 3): ([lambda m=m: O(m) for m in range(8, 11)], [lambda: O(11)]),
        }
        full = {0: 3, 1: 3, 2: 2, 3: 2, 4: 2, 5: 2, 6: 2, 7: 2, 8: 2}
        for hp in range(NF):
            for nq in range(NQB):
                reg, end = plan[(hp, nq)]
                fillers.extend(reg)
                end_fillers.extend(end)
                unit(hp, nq, slots=full if (hp, nq) == (0, 0) else None)
        # drain remaining fillers, then tail output projections
        while fillers or end_fillers:
            pump(1)
            pump_end()
        for mq in range(12, 16):
            emit_o_unit(mq)
        if debug:
            qt_dbg = nc.declare_dram_parameter("qt_dbg", [2 * NF * 128, S], bf16,
                                               isOutput=True)
            kt_dbg = nc.declare_dram_parameter("kt_dbg", [NF * 128, S], bf16,
                                               isOutput=True)
            v_dbg = nc.declare_dram_parameter("v_dbg", [NK * 128, VW], bf16,
                                              isOutput=True)
            cx_dbg = nc.declare_dram_parameter("cx_dbg", [NF * 128, S], bf16,
                                               isOutput=True)
            for h in range(2 * NF):
                nc.sync.dma_start(out=qt_dbg[h * 128:(h + 1) * 128, :], in_=qt_t[h][:])
            for m in range(NF):
                nc.sync.dma_start(out=kt_dbg[m * 128:(m + 1) * 128, :], in_=kt_t[m][:])
                nc.sync.dma_start(out=cx_dbg[m * 128:(m + 1) * 128, :], in_=ctxt_t[m][:])
            for mk in range(NK):
                nc.sync.dma_start(out=v_dbg[mk * 128:(mk + 1) * 128, :], in_=v_t[mk][:])
    nc.compile()
    return nc


def _get_nc():
    global _NC
    if _NC is None:
        _NC = _build()
    return _NC


def kernel(x, Wq, bq, Wk, bk, Wv, bv, Wo, bo, _trace=False):
    x = np.asarray(x, dtype=np.float32)
    Wq = np.asarray(Wq, dtype=np.float32)
    bq = np.asarray(bq, dtype=np.float32)
    Wk = np.asarray(Wk, dtype=np.float32)
    bk = np.asarray(bk, dtype=np.float32)
    Wv = np.asarray(Wv, dtype=np.float32)
    bv = np.asarray(bv, dtype=np.float32)
    Wo = np.asarray(Wo, dtype=np.float32)
    bo = np.asarray(bo, dtype=np.float32)

    nc = _get_nc()
    in_maps = []
    for c in range(NCORES):
        b = c // 2
        h0 = (c % 2) * HL
        in_maps.append({
            "xt": np.ascontiguousarray(x[b].T).astype(BF),
            "wq": np.ascontiguousarray(
                Wq[h0:h0 + HL].transpose(1, 0, 2).reshape(D, F)).astype(BF),
            "wk": np.ascontiguousarray(
                Wk[h0:h0 + HL].transpose(1, 0, 2).reshape(D, F)).astype(BF),
            "wv": np.ascontiguousarray(
                Wv[h0:h0 + HL].transpose(1, 0, 2).reshape(D, F)).astype(BF),
            "wo": np.ascontiguousarray(Wo[h0 * E:(h0 + HL) * E]).astype(BF),
            "bq": np.ascontiguousarray(bq[h0:h0 + HL].reshape(F, 1)),
            "bk": np.ascontiguousarray(bk[h0:h0 + HL].reshape(F, 1)),
            "bv": np.ascontiguousarray(bv[h0:h0 + HL].reshape(1, F)),
        })
    res = run_bass_kernel_spmd(nc, in_maps, list(range(NCORES)), trace=_trace)
    out = np.empty((B, S, D), np.float32)
    for b in range(B):
        out[b] = (res.results[2 * b]["y"].astype(np.float32)
                  + res.results[2 * b + 1]["y"].astype(np.float32)
                  + bo[None, :])
    if _trace:
        kernel.last_exec_time_ns = res.exec_time_ns
        kernel.last_results = res
    return out
